# revision 1
# baseline (speedup 1.0000x reference)
"""DeepSeek-V4 MLA sparse attention — Trainium2 Bass kernel, 8 NeuronCores.

Contract: kernel(**inputs) takes the FULL unsharded inputs
  q [512,64,576] f32, kv_cache [32768,576] f32,
  topk_indices [512,512] i32, attn_sink [64] f32
and returns the FULL output [512,64,512] f32.

Strategy (token/data-parallel per the sharding hint):
  - tokens sharded 8 ways (64/core); kv_cache replicated per core.

MODE "f16" (default, rel err ~1e-3 << 2e-2 gate):
  - KV cache host-converted to fp16 and padded to 640 cols (1280B rows,
    %256 for the DGE). q host-scaled, zero-padded to 640, d-major fp16
    [t,128p,5c,64h]; topk -> int16 SWDGE wrap.
  - device, per token pair (A,B):
      * gpsimd dma_gather(transpose=True): K^T lands directly in SBUF as
        [128 (d%128), 5 (d//128), 512 j] fp16 — no PE transposes and no
        PSUM drains to build K^T (the fp32 baseline couldn't use this:
        transpose-mode gather requires <=16-bit dtype).
      * V ([j, dv] layout for the PV rhs) rebuilt from K^T's latent
        chunks by 16 PE transposes per token; PSUM->SBUF drains split
        between ScalarE and DVE.
      * QK^T: fp16 matmuls column-tiled (token A -> psum partitions
        0-63, B -> 64-127), accumulating over the 5 d-chunks (zero-pad
        makes all chunks full 128 rows).
      * sink-softmax with NO max-subtraction (shift-invariant; logits
        ~N(0,1)): p = exp(s) fp16 + fp32 row-sum in one ScalarE op;
        denom/recip on DVE.
      * p^T via one [128,128] PE transpose per topk block.
      * PV: fp16 matmuls column-tiled over 4 topk blocks; out = pv *
        (1/denom) fused with the PSUM drain on DVE; fp32 store.

MODE "fp32": the original all-fp32 kernel (rel err ~3e-6), kept as a
fallback; normal (non-transposed) gather + PE-transposed K^T.
"""

import numpy as np
from contextlib import ExitStack, nullcontext

import concourse.mybir as mybir
import concourse.tile as tile
from concourse import bacc
from concourse.bass_utils import run_bass_kernel_spmd

F32 = mybir.dt.float32
F16 = mybir.dt.float16
I16 = mybir.dt.int16

T_FULL = 512
H = 64
D = 576
DV = 512
NKV = 32768
TOPK = 512
N_CORES = 8
T_LOC = T_FULL // N_CORES
SCALE = float(D) ** -0.5
NCH = 5       # d-chunks of 128 (576 zero-padded to 640)
DP = NCH * 128  # 640: padded row length (1280B fp16, %256 for the DGE)
NB = TOPK // 128  # topk blocks of 128

MODE = "f16"
SORT_IDX = False


def build_program_f16(t_loc=T_LOC, repeat=1, ablate=None, single_packet=True,
                      n_queues=2, pair_gather=False):
    # pair_gather=True (one 1024-idx SWDGE op per pair) halves the per-op
    # desc-gen fixed cost in the cost model, but hit repeated device-side
    # hangs ("mesh desynced") on HW — likely the 1024-descriptor SWDGE
    # ring carveout; kept off by default.
    nc = bacc.Bacc("TRN2", target_bir_lowering=False, debug=False,
                   num_swdge_queues=n_queues)
    q_t = nc.dram_tensor("q_t", [t_loc, 128, NCH, H], F16,
                         kind="ExternalInput")
    kv = nc.dram_tensor("kv", [NKV, DP], F16, kind="ExternalInput")
    idx = nc.dram_tensor("idx", [t_loc, 128, TOPK // 16], I16,
                         kind="ExternalInput")
    esink = nc.dram_tensor("esink", [128, 1], F32, kind="ExternalInput")
    ident_d = nc.dram_tensor("ident", [128, 128], F16, kind="ExternalInput")
    out = nc.dram_tensor("out", [t_loc, H, DV], F32, kind="ExternalOutput")

    out_flat = out.ap().rearrange("t h d -> (t h) d")

    with tile.TileContext(nc) as tc, ExitStack() as ctx:
        consts = ctx.enter_context(tc.tile_pool(name="consts", bufs=1))
        kq = ctx.enter_context(tc.tile_pool(name="kq", bufs=4))
        vp = ctx.enter_context(tc.tile_pool(name="vp", bufs=3))
        soft = ctx.enter_context(tc.tile_pool(name="soft", bufs=2))
        outp = ctx.enter_context(tc.tile_pool(name="outp", bufs=2))
        small = ctx.enter_context(tc.tile_pool(name="small", bufs=4))
        ps_vt = ctx.enter_context(
            tc.tile_pool(name="ps_vt", bufs=2, space="PSUM"))
        ps_sc = ctx.enter_context(
            tc.tile_pool(name="ps_sc", bufs=2, space="PSUM"))
        ps_pt = ctx.enter_context(
            tc.tile_pool(name="ps_pt", bufs=2, space="PSUM"))
        ps_pv = ctx.enter_context(
            tc.tile_pool(name="ps_pv", bufs=2, space="PSUM"))

        ident = consts.tile([128, 128], F16)
        nc.sync.dma_start(out=ident[:], in_=ident_d.ap())
        es_sb = consts.tile([128, 1], F32)
        nc.sync.dma_start(out=es_sb[:], in_=esink.ap())
        # Warmup transpose absorbs the identity-DMA wait up front.
        warm = ps_pt.tile([128, 128], F16, tag="ps_pt")
        nc.tensor.transpose(warm[:], ident[:], ident[:])
        if ablate == "gather":
            junk = consts.tile([128, DV], F32)
            for cc in range(4):
                nc.vector.tensor_copy(junk[:, cc * 128:(cc + 1) * 128],
                                      ident[:])
        kt0 = None
        if ablate == "compute":
            idx0 = consts.tile([128, TOPK // 16], I16)
            nc.sync.dma_start(out=idx0[:], in_=idx.ap()[0])
            kt0 = consts.tile([128, NCH, TOPK], F16)
            nc.gpsimd.dma_gather(
                out_ap=kt0[:], in_ap=kv.ap(), idxs_ap=idx0[:],
                num_idxs=TOPK, num_idxs_reg=TOPK, elem_size=DP,
                transpose=True, queue_num=0,
            )

        def load_token(t, queue):
            idx_sb = kq.tile([128, TOPK // 16], I16, tag="idx")
            nc.sync.dma_start(out=idx_sb[:], in_=idx.ap()[t])
            # K^T gathered directly: out[p, c, j] = kv[idx[j], c*128+p]
            if ablate == "compute":
                # compute-only floor: one shared gather done at startup
                # stands in for every per-token gather.
                q_sb = kq.tile([128, NCH, H], F16, tag="q")
                nc.sync.dma_start(out=q_sb[:], in_=q_t.ap()[t])
                return kt0, q_sb
            kt_sb = kq.tile([128, NCH, TOPK], F16, tag="kt")
            if n_queues == 4:
                half = TOPK // 2
                for h_i in range(2):
                    nc.gpsimd.dma_gather(
                        out_ap=kt_sb[:, :, h_i * half:(h_i + 1) * half],
                        in_ap=kv.ap(),
                        idxs_ap=idx_sb[:, h_i * (half // 16):
                                       (h_i + 1) * (half // 16)],
                        num_idxs=half,
                        num_idxs_reg=half,
                        elem_size=DP,
                        transpose=True,
                        single_packet=single_packet,
                        queue_num=2 * queue + h_i,
                    )
            else:
                nc.gpsimd.dma_gather(
                    out_ap=kt_sb[:],
                    in_ap=kv.ap(),
                    idxs_ap=idx_sb[:],
                    num_idxs=TOPK,
                    num_idxs_reg=TOPK,
                    elem_size=DP,
                    transpose=True,
                    single_packet=single_packet,
                    queue_num=queue,
                )
            q_sb = kq.tile([128, NCH, H], F16, tag="q")
            nc.sync.dma_start(out=q_sb[:], in_=q_t.ap()[t])
            return kt_sb, q_sb

        def build_v(kt_sb, tok, j0=0):
            # V[j, dv] from the latent chunks of K^T via PE transposes.
            v_sb = vp.tile([128, NB, DV], F16, tag="v")
            for b in range(NB):
                vps = ps_vt.tile([128, DV], F16, tag="ps_vt")
                for c in range(4):
                    nc.tensor.transpose(
                        vps[:, c * 128:(c + 1) * 128],
                        kt_sb[:, c, j0 + b * 128:j0 + (b + 1) * 128],
                        ident[:],
                    )
                if (tok * NB + b) % 2 == 0:
                    nc.scalar.copy(v_sb[:, b, :], vps[:])
                else:
                    nc.vector.tensor_copy(v_sb[:, b, :], vps[:])
            return v_sb

        def load_pair(tA):
            # One SWDGE gather for both tokens (1024 idx): halves the
            # per-op desc-gen fixed cost (~1us) vs two 512-idx gathers.
            idx_sb = kq.tile([128, 2 * (TOPK // 16)], I16, tag="idx")
            nc.sync.dma_start(out=idx_sb[:, 0:TOPK // 16],
                              in_=idx.ap()[tA])
            nc.sync.dma_start(out=idx_sb[:, TOPK // 16:],
                              in_=idx.ap()[tA + 1])
            kt2 = kq.tile([128, NCH, 2 * TOPK], F16, tag="kt2")
            nc.gpsimd.dma_gather(
                out_ap=kt2[:],
                in_ap=kv.ap(),
                idxs_ap=idx_sb[:],
                num_idxs=2 * TOPK,
                num_idxs_reg=2 * TOPK,
                elem_size=DP,
                transpose=True,
                single_packet=single_packet,
                queue_num=(tA // 2) % min(n_queues, 2),
            )
            qA = kq.tile([128, NCH, H], F16, tag="q")
            nc.sync.dma_start(out=qA[:], in_=q_t.ap()[tA])
            qB = kq.tile([128, NCH, H], F16, tag="qb")
            nc.sync.dma_start(out=qB[:], in_=q_t.ap()[tA + 1])
            return kt2, qA, qB

        def pair_body(tA):
            if pair_gather:
                kt2, qA, qB = load_pair(tA)
                ktA = ktB = kt2
                offA, offB = 0, TOPK
            else:
                ktA, qA = load_token(tA, 0)
                ktB, qB = load_token(tA + 1, 1)
                offA = offB = 0
            if ablate == "gather":
                # DMA-only floor: loads + a junk store, no compute.
                nc.sync.dma_start(
                    out=out_flat[tA * H:tA * H + 128, :], in_=junk[:])
                return
            vA = build_v(ktA, 0, offA)

            sc = ps_sc.tile([128, TOPK], F32, tag="sc")
            for c in range(NCH):
                st, sp = (c == 0), (c == NCH - 1)
                nc.tensor.matmul(
                    sc[0:64, :], lhsT=qA[:, c, :],
                    rhs=ktA[:, c, offA:offA + TOPK],
                    start=st, stop=sp, tile_position=(0, 0),
                    skip_group_check=True,
                )
                nc.tensor.matmul(
                    sc[64:128, :], lhsT=qB[:, c, :],
                    rhs=ktB[:, c, offB:offB + TOPK],
                    start=st, stop=sp, tile_position=(0, 64),
                    skip_group_check=True,
                )

            vB = build_v(ktB, 1, offB)

            p_sb = soft.tile([128, TOPK], F16, tag="p")
            sum_p = small.tile([128, 1], F32, tag="sum")
            nc.scalar.activation(
                p_sb[:], sc[:], mybir.ActivationFunctionType.Exp,
                accum_out=sum_p[:],
            )
            den = small.tile([128, 1], F32, tag="den")
            nc.vector.tensor_add(den[:], sum_p[:], es_sb[:])
            rec = small.tile([128, 1], F32, tag="rec")
            nc.vector.reciprocal(rec[:], den[:])

            pt_sb = soft.tile([128, NB, 128], F16, tag="pt")
            for b in range(NB):
                pst = ps_pt.tile([128, 128], F16, tag="ps_pt")
                nc.tensor.transpose(
                    pst[:], p_sb[:, b * 128:(b + 1) * 128], ident[:])
                nc.vector.tensor_copy(pt_sb[:, b, :], pst[:])

            pv = ps_pv.tile([128, DV], F32, tag="pv")
            for b in range(NB):
                st, sp = (b == 0), (b == NB - 1)
                nc.tensor.matmul(
                    pv[0:64, :], lhsT=pt_sb[:, b, 0:64], rhs=vA[:, b, :],
                    start=st, stop=sp, tile_position=(0, 0),
                    skip_group_check=True,
                )
                nc.tensor.matmul(
                    pv[64:128, :], lhsT=pt_sb[:, b, 64:128], rhs=vB[:, b, :],
                    start=st, stop=sp, tile_position=(0, 64),
                    skip_group_check=True,
                )

            o_sb = outp.tile([128, DV], F32, tag="o")
            nc.vector.tensor_scalar_mul(o_sb[:], pv[:], rec[:])
            nc.sync.dma_start(
                out=out_flat[tA * H:tA * H + 128, :], in_=o_sb[:])

        loop_cm = tc.For_i(0, repeat, 1) if repeat > 1 else nullcontext()
        with loop_cm:
            for i in range(t_loc // 2):
                pair_body(2 * i)

    nc.compile()
    return nc


def build_program_fp32(t_loc=T_LOC, repeat=1):
    """Original all-fp32 kernel (rel err ~3e-6); normal gather + PE K^T."""
    nc = bacc.Bacc("TRN2", target_bir_lowering=False, debug=False)
    q_t = nc.dram_tensor("q_t", [t_loc, 128, NCH, H], F32,
                         kind="ExternalInput")
    kv = nc.dram_tensor("kv", [NKV, D], F32, kind="ExternalInput")
    idx = nc.dram_tensor("idx", [t_loc, 128, TOPK // 16], I16,
                         kind="ExternalInput")
    esink = nc.dram_tensor("esink", [128, 1], F32, kind="ExternalInput")
    ident_d = nc.dram_tensor("ident", [128, 128], F32, kind="ExternalInput")
    out = nc.dram_tensor("out", [t_loc, H, DV], F32, kind="ExternalOutput")

    out_flat = out.ap().rearrange("t h d -> (t h) d")

    with tile.TileContext(nc) as tc, ExitStack() as ctx:
        consts = ctx.enter_context(tc.tile_pool(name="consts", bufs=1))
        kq = ctx.enter_context(tc.tile_pool(name="kq", bufs=5))
        ktp = ctx.enter_context(tc.tile_pool(name="ktp", bufs=3))
        soft = ctx.enter_context(tc.tile_pool(name="soft", bufs=2))
        outp = ctx.enter_context(tc.tile_pool(name="outp", bufs=2))
        small = ctx.enter_context(tc.tile_pool(name="small", bufs=4))
        ps_kt = ctx.enter_context(
            tc.tile_pool(name="ps_kt", bufs=2, space="PSUM"))
        ps_sc = ctx.enter_context(
            tc.tile_pool(name="ps_sc", bufs=2, space="PSUM"))
        ps_pt = ctx.enter_context(
            tc.tile_pool(name="ps_pt", bufs=2, space="PSUM"))
        ps_pv = ctx.enter_context(
            tc.tile_pool(name="ps_pv", bufs=2, space="PSUM"))

        ident = consts.tile([128, 128], F32)
        nc.sync.dma_start(out=ident[:], in_=ident_d.ap())
        es_sb = consts.tile([128, 1], F32)
        nc.sync.dma_start(out=es_sb[:], in_=esink.ap())
        warm = ps_pt.tile([128, 128], F32, tag="ps_pt")
        nc.tensor.transpose(warm[:], ident[:], ident[:])

        def load_token(t):
            idx_sb = kq.tile([128, TOPK // 16], I16, tag="idx")
            nc.sync.dma_start(out=idx_sb[:], in_=idx.ap()[t])
            k_sb = kq.tile([128, NB, D], F32, tag="k")
            nc.gpsimd.dma_gather(
                out_ap=k_sb[:],
                in_ap=kv.ap(),
                idxs_ap=idx_sb[:],
                num_idxs=TOPK,
                num_idxs_reg=TOPK,
                elem_size=D,
            )
            q_sb = kq.tile([128, NCH, H], F32, tag="q")
            nc.sync.dma_start(out=q_sb[:], in_=q_t.ap()[t])
            q_act = kq.tile([128, NCH, H], F32, tag="qa")
            nc.scalar.copy(q_act[:], q_sb[:])
            return k_sb, q_act

        def build_kt(k_sb):
            kt_sb = ktp.tile([128, NCH, TOPK], F32, tag="kt")
            for c in range(NCH):
                pp = 128 if c < 4 else D - 512
                pst = ps_kt.tile([128, TOPK], F32, tag="ps_kt")
                for b in range(NB):
                    nc.tensor.transpose(
                        pst[:pp, b * 128:(b + 1) * 128],
                        k_sb[:, b, c * 128:c * 128 + pp],
                        ident[:],
                    )
                nc.scalar.copy(kt_sb[:pp, c, :], pst[:pp, :])
            return kt_sb

        def pair_body(tA):
            kA, qA = load_token(tA)
            kB, qB = load_token(tA + 1)
            ktA = build_kt(kA)
            ktB = build_kt(kB)

            sc = ps_sc.tile([128, TOPK], F32, tag="sc")
            for c in range(NCH):
                kk = 128 if c < 4 else D - 512
                st, sp = (c == 0), (c == NCH - 1)
                nc.tensor.matmul(
                    sc[0:64, :], lhsT=qA[:kk, c, :], rhs=ktA[:kk, c, :],
                    start=st, stop=sp, tile_position=(0, 0),
                    skip_group_check=True,
                )
                nc.tensor.matmul(
                    sc[64:128, :], lhsT=qB[:kk, c, :], rhs=ktB[:kk, c, :],
                    start=st, stop=sp, tile_position=(0, 64),
                    skip_group_check=True,
                )

            p_sb = soft.tile([128, TOPK], F32, tag="p")
            sum_p = small.tile([128, 1], F32, tag="sum")
            nc.scalar.activation(
                p_sb[:], sc[:], mybir.ActivationFunctionType.Exp,
                accum_out=sum_p[:],
            )
            den = small.tile([128, 1], F32, tag="den")
            nc.vector.tensor_add(den[:], sum_p[:], es_sb[:])
            rec = small.tile([128, 1], F32, tag="rec")
            nc.vector.reciprocal(rec[:], den[:])

            pt_sb = soft.tile([128, NB, 128], F32, tag="pt")
            for b in range(NB):
                pst = ps_pt.tile([128, 128], F32, tag="ps_pt")
                nc.tensor.transpose(
                    pst[:], p_sb[:, b * 128:(b + 1) * 128], ident[:])
                nc.vector.tensor_copy(pt_sb[:, b, :], pst[:])

            pv = ps_pv.tile([128, DV], F32, tag="pv")
            for b in range(NB):
                st, sp = (b == 0), (b == NB - 1)
                nc.tensor.matmul(
                    pv[0:64, :], lhsT=pt_sb[:, b, 0:64], rhs=kA[:, b, 0:DV],
                    start=st, stop=sp, tile_position=(0, 0),
                    skip_group_check=True,
                )
                nc.tensor.matmul(
                    pv[64:128, :], lhsT=pt_sb[:, b, 64:128],
                    rhs=kB[:, b, 0:DV],
                    start=st, stop=sp, tile_position=(0, 64),
                    skip_group_check=True,
                )

            o_sb = outp.tile([128, DV], F32, tag="o")
            nc.vector.tensor_scalar_mul(o_sb[:], pv[:], rec[:])
            nc.sync.dma_start(
                out=out_flat[tA * H:tA * H + 128, :], in_=o_sb[:])

        loop_cm = tc.For_i(0, repeat, 1) if repeat > 1 else nullcontext()
        with loop_cm:
            for i in range(t_loc // 2):
                pair_body(2 * i)

    nc.compile()
    return nc


def build_program(t_loc=T_LOC, repeat=1, mode=MODE):
    if mode == "f16":
        return build_program_f16(t_loc, repeat)
    assert mode == "fp32"
    return build_program_fp32(t_loc, repeat)


# ---------------- host-side prep ----------------

def prep_shared(kv_cache, attn_sink, mode=MODE):
    """Per-run, core-independent host prep (replicated to every core)."""
    es = np.exp(np.asarray(attn_sink, np.float64)).astype(np.float32)
    esink = np.ascontiguousarray(np.tile(es, 2)[:, None])
    if mode == "f16":
        kv = np.zeros((NKV, DP), np.float16)
        kv[:, :D] = np.asarray(kv_cache, np.float32).astype(np.float16)
        ident = np.eye(128, dtype=np.float16)
    else:
        kv = np.ascontiguousarray(np.asarray(kv_cache, np.float32))
        ident = np.eye(128, dtype=np.float32)
    return {"kv": kv, "esink": esink, "ident": ident}


def prep_core_inputs(q, shared, topk_indices, core, t_loc=T_LOC, mode=MODE):
    t0 = core * t_loc
    qs = (np.asarray(q[t0:t0 + t_loc]) * SCALE).astype(np.float32)
    qpad = np.zeros((t_loc, H, DP), np.float32)
    qpad[:, :, :D] = qs
    qtr = qpad.reshape(t_loc, H, NCH, 128).transpose(0, 3, 2, 1)
    if mode == "f16":
        q_t = np.ascontiguousarray(qtr.astype(np.float16))
    else:
        q_t = np.ascontiguousarray(qtr)

    tk = np.asarray(topk_indices[t0:t0 + t_loc])
    if SORT_IDX:
        # softmax/PV are order-invariant over the topk axis, so sorting
        # is legal; whether it helps HBM locality is measured, not assumed.
        tk = np.sort(tk, axis=-1)
    tk = tk.astype(np.int16)
    wrap = tk.reshape(t_loc, TOPK // 16, 16).transpose(0, 2, 1)
    idx = np.ascontiguousarray(np.tile(wrap, (1, 8, 1)))

    return {"q_t": q_t, "kv": shared["kv"], "idx": idx,
            "esink": shared["esink"], "ident": shared["ident"]}


_PROGRAM_CACHE = {}


def _get_program(t_loc, mode=MODE):
    key = (t_loc, mode)
    if key not in _PROGRAM_CACHE:
        _PROGRAM_CACHE[key] = build_program(t_loc, mode=mode)
    return _PROGRAM_CACHE[key]


def run(q, kv_cache, topk_indices, attn_sink, trace=False, mode=MODE):
    nc = _get_program(T_LOC, mode)
    shared = prep_shared(kv_cache, attn_sink, mode)
    in_maps = [
        prep_core_inputs(q, shared, topk_indices, c, mode=mode)
        for c in range(N_CORES)
    ]
    res = run_bass_kernel_spmd(nc, in_maps, list(range(N_CORES)),
                               trace=trace)
    out = np.concatenate([res.results[c]["out"] for c in range(N_CORES)],
                         axis=0)
    return out, res


def kernel(q, kv_cache, topk_indices, attn_sink):
    out, _ = run(q, kv_cache, topk_indices, attn_sink, trace=False)
    return out.astype(np.float32)



# revision 10
# speedup vs baseline: 1.7171x; 1.7171x over previous
"""DeepSeek-V4 MLA sparse attention — Trainium2 Bass kernel, 8 NeuronCores.

Contract: kernel(**inputs) takes the FULL unsharded inputs
  q [512,64,576] f32, kv_cache [32768,576] f32,
  topk_indices [512,512] i32, attn_sink [64] f32
and returns the FULL output [512,64,512] f32.

Strategy (token/data-parallel per the sharding hint):
  - tokens sharded 8 ways (64/core); kv_cache replicated per core.

MODE "f16" (default, rel err ~1e-3 << 2e-2 gate):
  - KV cache host-converted to fp16 and padded to 640 cols (1280B rows,
    %256 for the DGE). q host-scaled, zero-padded to 640, d-major fp16
    [t,128p,5c,64h]; topk -> int16 SWDGE wrap.
  - device, per token pair (A,B):
      * gpsimd dma_gather(transpose=True): K^T lands directly in SBUF as
        [128 (d%128), 5 (d//128), 512 j] fp16 — no PE transposes and no
        PSUM drains to build K^T (the fp32 baseline couldn't use this:
        transpose-mode gather requires <=16-bit dtype).
      * V ([j, dv] layout for the PV rhs) rebuilt from K^T's latent
        chunks by 16 PE transposes per token; PSUM->SBUF drains split
        between ScalarE and DVE.
      * QK^T: fp16 matmuls column-tiled (token A -> psum partitions
        0-63, B -> 64-127), accumulating over the 5 d-chunks (zero-pad
        makes all chunks full 128 rows).
      * sink-softmax with NO max-subtraction (shift-invariant; logits
        ~N(0,1)): p = exp(s) fp16 + fp32 row-sum in one ScalarE op;
        denom/recip on DVE.
      * p^T via one [128,128] PE transpose per topk block.
      * PV: fp16 matmuls column-tiled over 4 topk blocks; out = pv *
        (1/denom) fused with the PSUM drain on DVE; fp32 store.

MODE "fp32": the original all-fp32 kernel (rel err ~3e-6), kept as a
fallback; normal (non-transposed) gather + PE-transposed K^T.
"""

import numpy as np
from contextlib import ExitStack, nullcontext

import concourse.mybir as mybir
import concourse.tile as tile
from concourse import bacc
from concourse.bass_utils import run_bass_kernel_spmd

F32 = mybir.dt.float32
F16 = mybir.dt.float16
I16 = mybir.dt.int16

T_FULL = 512
H = 64
D = 576
DV = 512
NKV = 32768
TOPK = 512
N_CORES = 8
T_LOC = T_FULL // N_CORES
SCALE = float(D) ** -0.5
NCH = 5       # d-chunks of 128 (576 zero-padded to 640)
DP = NCH * 128  # 640: padded row length (1280B fp16, %256 for the DGE)
NB = TOPK // 128  # topk blocks of 128

MODE = "v2"
SORT_IDX = False


def build_program_f16(t_loc=T_LOC, repeat=1, ablate=None, single_packet=True,
                      n_queues=4, pair_gather=False, kq_bufs=4):
    # pair_gather=True (one 1024-idx SWDGE op per pair) halves the per-op
    # desc-gen fixed cost in the cost model, but hit repeated device-side
    # hangs ("mesh desynced") on HW — likely the 1024-descriptor SWDGE
    # ring carveout; kept off by default.
    nc = bacc.Bacc("TRN2", target_bir_lowering=False, debug=False,
                   num_swdge_queues=n_queues)
    q_t = nc.dram_tensor("q_t", [t_loc, 128, NCH, H], F16,
                         kind="ExternalInput")
    kv = nc.dram_tensor("kv", [NKV, DP], F16, kind="ExternalInput")
    idx = nc.dram_tensor("idx", [t_loc, 128, TOPK // 16], I16,
                         kind="ExternalInput")
    esink = nc.dram_tensor("esink", [128, 1], F32, kind="ExternalInput")
    ident_d = nc.dram_tensor("ident", [128, 128], F16, kind="ExternalInput")
    out = nc.dram_tensor("out", [t_loc, H, DV], F32, kind="ExternalOutput")

    out_flat = out.ap().rearrange("t h d -> (t h) d")

    with tile.TileContext(nc) as tc, ExitStack() as ctx:
        consts = ctx.enter_context(tc.tile_pool(name="consts", bufs=1))
        kq = ctx.enter_context(tc.tile_pool(name="kq", bufs=kq_bufs))
        vp = ctx.enter_context(tc.tile_pool(name="vp", bufs=3))
        soft = ctx.enter_context(tc.tile_pool(name="soft", bufs=2))
        outp = ctx.enter_context(tc.tile_pool(name="outp", bufs=2))
        small = ctx.enter_context(tc.tile_pool(name="small", bufs=4))
        ps_vt = ctx.enter_context(
            tc.tile_pool(name="ps_vt", bufs=2, space="PSUM"))
        ps_sc = ctx.enter_context(
            tc.tile_pool(name="ps_sc", bufs=2, space="PSUM"))
        ps_pt = ctx.enter_context(
            tc.tile_pool(name="ps_pt", bufs=2, space="PSUM"))
        ps_pv = ctx.enter_context(
            tc.tile_pool(name="ps_pv", bufs=2, space="PSUM"))

        ident = consts.tile([128, 128], F16)
        nc.sync.dma_start(out=ident[:], in_=ident_d.ap())
        es_sb = consts.tile([128, 1], F32)
        nc.sync.dma_start(out=es_sb[:], in_=esink.ap())
        # Warmup transpose absorbs the identity-DMA wait up front.
        warm = ps_pt.tile([128, 128], F16, tag="ps_pt")
        nc.tensor.transpose(warm[:], ident[:], ident[:])
        if ablate == "gather":
            junk = consts.tile([128, DV], F32)
            for cc in range(4):
                nc.vector.tensor_copy(junk[:, cc * 128:(cc + 1) * 128],
                                      ident[:])
        kt0 = None
        if ablate == "compute":
            idx0 = consts.tile([128, TOPK // 16], I16)
            nc.sync.dma_start(out=idx0[:], in_=idx.ap()[0])
            kt0 = consts.tile([128, NCH, TOPK], F16)
            nc.gpsimd.dma_gather(
                out_ap=kt0[:], in_ap=kv.ap(), idxs_ap=idx0[:],
                num_idxs=TOPK, num_idxs_reg=TOPK, elem_size=DP,
                transpose=True, queue_num=0,
            )

        def load_token(t):
            idx_sb = kq.tile([128, TOPK // 16], I16, tag="idx")
            nc.sync.dma_start(out=idx_sb[:], in_=idx.ap()[t])
            # K^T gathered directly: out[p, c, j] = kv[idx[j], c*128+p]
            if ablate == "compute":
                # compute-only floor: one shared gather done at startup
                # stands in for every per-token gather.
                q_sb = kq.tile([128, NCH, H], F16, tag="q")
                nc.sync.dma_start(out=q_sb[:], in_=q_t.ap()[t])
                return kt0, q_sb
            kt_sb = kq.tile([128, NCH, TOPK], F16, tag="kt")
            nc.gpsimd.dma_gather(
                out_ap=kt_sb[:],
                in_ap=kv.ap(),
                idxs_ap=idx_sb[:],
                num_idxs=TOPK,
                num_idxs_reg=TOPK,
                elem_size=DP,
                transpose=True,
                single_packet=single_packet,
                queue_num=t % n_queues,
            )
            q_sb = kq.tile([128, NCH, H], F16, tag="q")
            nc.sync.dma_start(out=q_sb[:], in_=q_t.ap()[t])
            return kt_sb, q_sb

        def build_v(kt_sb, tok, j0=0):
            # V[j, dv] from the latent chunks of K^T via PE transposes.
            v_sb = vp.tile([128, NB, DV], F16, tag="v")
            for b in range(NB):
                vps = ps_vt.tile([128, DV], F16, tag="ps_vt")
                for c in range(4):
                    nc.tensor.transpose(
                        vps[:, c * 128:(c + 1) * 128],
                        kt_sb[:, c, j0 + b * 128:j0 + (b + 1) * 128],
                        ident[:],
                    )
                if (tok * NB + b) % 2 == 0:
                    nc.scalar.copy(v_sb[:, b, :], vps[:])
                else:
                    nc.vector.tensor_copy(v_sb[:, b, :], vps[:])
            return v_sb

        def load_pair(tA):
            # One SWDGE gather for both tokens (1024 idx): halves the
            # per-op desc-gen fixed cost (~1us) vs two 512-idx gathers.
            idx_sb = kq.tile([128, 2 * (TOPK // 16)], I16, tag="idx")
            nc.sync.dma_start(out=idx_sb[:, 0:TOPK // 16],
                              in_=idx.ap()[tA])
            nc.sync.dma_start(out=idx_sb[:, TOPK // 16:],
                              in_=idx.ap()[tA + 1])
            kt2 = kq.tile([128, NCH, 2 * TOPK], F16, tag="kt2")
            nc.gpsimd.dma_gather(
                out_ap=kt2[:],
                in_ap=kv.ap(),
                idxs_ap=idx_sb[:],
                num_idxs=2 * TOPK,
                num_idxs_reg=2 * TOPK,
                elem_size=DP,
                transpose=True,
                single_packet=single_packet,
                queue_num=(tA // 2) % min(n_queues, 2),
            )
            qA = kq.tile([128, NCH, H], F16, tag="q")
            nc.sync.dma_start(out=qA[:], in_=q_t.ap()[tA])
            qB = kq.tile([128, NCH, H], F16, tag="qb")
            nc.sync.dma_start(out=qB[:], in_=q_t.ap()[tA + 1])
            return kt2, qA, qB

        def pair_body(tA):
            if pair_gather:
                kt2, qA, qB = load_pair(tA)
                ktA = ktB = kt2
                offA, offB = 0, TOPK
            else:
                ktA, qA = load_token(tA)
                ktB, qB = load_token(tA + 1)
                offA = offB = 0
            if ablate == "gather":
                # DMA-only floor: loads + a junk store, no compute.
                nc.sync.dma_start(
                    out=out_flat[tA * H:tA * H + 128, :], in_=junk[:])
                return
            vA = build_v(ktA, 0, offA)

            sc = ps_sc.tile([128, TOPK], F32, tag="sc")
            for c in range(NCH):
                st, sp = (c == 0), (c == NCH - 1)
                nc.tensor.matmul(
                    sc[0:64, :], lhsT=qA[:, c, :],
                    rhs=ktA[:, c, offA:offA + TOPK],
                    start=st, stop=sp, tile_position=(0, 0),
                    skip_group_check=True,
                )
                nc.tensor.matmul(
                    sc[64:128, :], lhsT=qB[:, c, :],
                    rhs=ktB[:, c, offB:offB + TOPK],
                    start=st, stop=sp, tile_position=(0, 64),
                    skip_group_check=True,
                )

            vB = build_v(ktB, 1, offB)

            p_sb = soft.tile([128, TOPK], F16, tag="p")
            sum_p = small.tile([128, 1], F32, tag="sum")
            nc.scalar.activation(
                p_sb[:], sc[:], mybir.ActivationFunctionType.Exp,
                accum_out=sum_p[:],
            )
            den = small.tile([128, 1], F32, tag="den")
            nc.vector.tensor_add(den[:], sum_p[:], es_sb[:])
            rec = small.tile([128, 1], F32, tag="rec")
            nc.vector.reciprocal(rec[:], den[:])

            pt_sb = soft.tile([128, NB, 128], F16, tag="pt")
            for b in range(NB):
                pst = ps_pt.tile([128, 128], F16, tag="ps_pt")
                nc.tensor.transpose(
                    pst[:], p_sb[:, b * 128:(b + 1) * 128], ident[:])
                nc.vector.tensor_copy(pt_sb[:, b, :], pst[:])

            pv = ps_pv.tile([128, DV], F32, tag="pv")
            for b in range(NB):
                st, sp = (b == 0), (b == NB - 1)
                nc.tensor.matmul(
                    pv[0:64, :], lhsT=pt_sb[:, b, 0:64], rhs=vA[:, b, :],
                    start=st, stop=sp, tile_position=(0, 0),
                    skip_group_check=True,
                )
                nc.tensor.matmul(
                    pv[64:128, :], lhsT=pt_sb[:, b, 64:128], rhs=vB[:, b, :],
                    start=st, stop=sp, tile_position=(0, 64),
                    skip_group_check=True,
                )

            o_sb = outp.tile([128, DV], F32, tag="o")
            nc.vector.tensor_scalar_mul(o_sb[:], pv[:], rec[:])
            nc.sync.dma_start(
                out=out_flat[tA * H:tA * H + 128, :], in_=o_sb[:])

        loop_cm = tc.For_i(0, repeat, 1) if repeat > 1 else nullcontext()
        with loop_cm:
            for i in range(t_loc // 2):
                pair_body(2 * i)

    nc.compile()
    return nc


def build_program_v2(t_loc=T_LOC, repeat=1, n_queues=4, kq_bufs=6,
                     single_packet=True, out_f16=True, vp_bufs=3):
    """Restructured f16 kernel.

    vs f16 v1: gathers rotate over 4 SWDGE queues (the 512-desc ops are
    ring-throughput-bound; 4 rings beat 2 by ~18% in isolation); PE order
    is [vA, QK, vB, pT, PV] so vA fills the gather-B wait and vB covers
    the exp latency; exp is chunked per topk block so p^T transposes
    start as soon as their block's exp lands; output stored f16 (host
    upcasts) halving store traffic.
    """
    nc = bacc.Bacc("TRN2", target_bir_lowering=False, debug=False,
                   num_swdge_queues=n_queues)
    q_t = nc.dram_tensor("q_t", [t_loc, 128, NCH, H], F16,
                         kind="ExternalInput")
    kv = nc.dram_tensor("kv", [NKV, DP], F16, kind="ExternalInput")
    idx = nc.dram_tensor("idx", [t_loc, 128, TOPK // 16], I16,
                         kind="ExternalInput")
    esink = nc.dram_tensor("esink", [128, 1], F32, kind="ExternalInput")
    ident_d = nc.dram_tensor("ident", [128, 128], F16, kind="ExternalInput")
    ODT = F16 if out_f16 else F32
    out = nc.dram_tensor("out", [t_loc, H, DV], ODT, kind="ExternalOutput")

    out_flat = out.ap().rearrange("t h d -> (t h) d")

    with tile.TileContext(nc) as tc, ExitStack() as ctx:
        consts = ctx.enter_context(tc.tile_pool(name="consts", bufs=1))
        kq = ctx.enter_context(tc.tile_pool(name="kq", bufs=kq_bufs))
        vp = ctx.enter_context(tc.tile_pool(name="vp", bufs=vp_bufs))
        soft = ctx.enter_context(tc.tile_pool(name="soft", bufs=2))
        outp = ctx.enter_context(tc.tile_pool(name="outp", bufs=2))
        small = ctx.enter_context(tc.tile_pool(name="small", bufs=4))
        ps_vt = ctx.enter_context(
            tc.tile_pool(name="ps_vt", bufs=2, space="PSUM"))
        ps_sc = ctx.enter_context(
            tc.tile_pool(name="ps_sc", bufs=2, space="PSUM"))
        ps_pt = ctx.enter_context(
            tc.tile_pool(name="ps_pt", bufs=2, space="PSUM"))
        ps_pv = ctx.enter_context(
            tc.tile_pool(name="ps_pv", bufs=2, space="PSUM"))

        ident = consts.tile([128, 128], F16)
        nc.sync.dma_start(out=ident[:], in_=ident_d.ap())
        es_sb = consts.tile([128, 1], F32)
        nc.sync.dma_start(out=es_sb[:], in_=esink.ap())
        warm = ps_pt.tile([128, 128], F16, tag="ps_pt")
        nc.tensor.transpose(warm[:], ident[:], ident[:])

        def load_token(t):
            idx_sb = kq.tile([128, TOPK // 16], I16, tag="idx")
            nc.sync.dma_start(out=idx_sb[:], in_=idx.ap()[t])
            kt_sb = kq.tile([128, NCH, TOPK], F16, tag="kt")
            nc.gpsimd.dma_gather(
                out_ap=kt_sb[:],
                in_ap=kv.ap(),
                idxs_ap=idx_sb[:],
                num_idxs=TOPK,
                num_idxs_reg=TOPK,
                elem_size=DP,
                transpose=True,
                single_packet=single_packet,
                queue_num=t % n_queues,
            )
            q_sb = kq.tile([128, NCH, H], F16, tag="q")
            nc.sync.dma_start(out=q_sb[:], in_=q_t.ap()[t])
            return kt_sb, q_sb

        def build_v(kt_sb, tok):
            v_sb = vp.tile([128, NB, DV], F16, tag="v")
            for b in range(NB):
                vps = ps_vt.tile([128, DV], F16, tag="ps_vt")
                for c in range(4):
                    nc.tensor.transpose(
                        vps[:, c * 128:(c + 1) * 128],
                        kt_sb[:, c, b * 128:(b + 1) * 128],
                        ident[:],
                    )
                if (tok * NB + b) % 2 == 0:
                    nc.scalar.copy(v_sb[:, b, :], vps[:])
                else:
                    nc.vector.tensor_copy(v_sb[:, b, :], vps[:])
            return v_sb

        def pair_body(tA):
            ktA, qA = load_token(tA)
            ktB, qB = load_token(tA + 1)

            vA = build_v(ktA, 0)   # PE fills the gather-B window

            sc = ps_sc.tile([128, TOPK], F32, tag="sc")
            for c in range(NCH):
                st, sp = (c == 0), (c == NCH - 1)
                nc.tensor.matmul(
                    sc[0:64, :], lhsT=qA[:, c, :], rhs=ktA[:, c, :],
                    start=st, stop=sp, tile_position=(0, 0),
                    skip_group_check=True,
                )
                nc.tensor.matmul(
                    sc[64:128, :], lhsT=qB[:, c, :], rhs=ktB[:, c, :],
                    start=st, stop=sp, tile_position=(0, 64),
                    skip_group_check=True,
                )

            # exp per topk block; partial row-sums land in one [128, NB]
            p_sb = soft.tile([128, NB, 128], F16, tag="p")
            sums = small.tile([128, NB], F32, tag="sums")
            for b in range(NB):
                nc.scalar.activation(
                    p_sb[:, b, :], sc[:, b * 128:(b + 1) * 128],
                    mybir.ActivationFunctionType.Exp,
                    accum_out=sums[:, b:b + 1],
                )

            vB = build_v(ktB, 1)   # PE covers the exp latency

            pt_sb = soft.tile([128, NB, 128], F16, tag="pt")
            for b in range(NB):
                pst = ps_pt.tile([128, 128], F16, tag="ps_pt")
                nc.tensor.transpose(
                    pst[:], p_sb[:, b, :], ident[:])
                nc.vector.tensor_copy(pt_sb[:, b, :], pst[:])

            den = small.tile([128, 1], F32, tag="den")
            nc.vector.tensor_reduce(
                den[:], sums[:], axis=mybir.AxisListType.X,
                op=mybir.AluOpType.add)
            den2 = small.tile([128, 1], F32, tag="den2")
            nc.vector.tensor_add(den2[:], den[:], es_sb[:])
            rec = small.tile([128, 1], F32, tag="rec")
            nc.vector.reciprocal(rec[:], den2[:])

            pv = ps_pv.tile([128, DV], F32, tag="pv")
            for b in range(NB):
                st, sp = (b == 0), (b == NB - 1)
                nc.tensor.matmul(
                    pv[0:64, :], lhsT=pt_sb[:, b, 0:64], rhs=vA[:, b, :],
                    start=st, stop=sp, tile_position=(0, 0),
                    skip_group_check=True,
                )
                nc.tensor.matmul(
                    pv[64:128, :], lhsT=pt_sb[:, b, 64:128], rhs=vB[:, b, :],
                    start=st, stop=sp, tile_position=(0, 64),
                    skip_group_check=True,
                )

            o_sb = outp.tile([128, DV], ODT, tag="o")
            nc.vector.tensor_scalar_mul(o_sb[:], pv[:], rec[:])
            nc.sync.dma_start(
                out=out_flat[tA * H:tA * H + 128, :], in_=o_sb[:])

        loop_cm = tc.For_i(0, repeat, 1) if repeat > 1 else nullcontext()
        with loop_cm:
            for i in range(t_loc // 2):
                pair_body(2 * i)

    nc.compile()
    return nc


def build_program_fp32(t_loc=T_LOC, repeat=1):
    """Original all-fp32 kernel (rel err ~3e-6); normal gather + PE K^T."""
    nc = bacc.Bacc("TRN2", target_bir_lowering=False, debug=False)
    q_t = nc.dram_tensor("q_t", [t_loc, 128, NCH, H], F32,
                         kind="ExternalInput")
    kv = nc.dram_tensor("kv", [NKV, D], F32, kind="ExternalInput")
    idx = nc.dram_tensor("idx", [t_loc, 128, TOPK // 16], I16,
                         kind="ExternalInput")
    esink = nc.dram_tensor("esink", [128, 1], F32, kind="ExternalInput")
    ident_d = nc.dram_tensor("ident", [128, 128], F32, kind="ExternalInput")
    out = nc.dram_tensor("out", [t_loc, H, DV], F32, kind="ExternalOutput")

    out_flat = out.ap().rearrange("t h d -> (t h) d")

    with tile.TileContext(nc) as tc, ExitStack() as ctx:
        consts = ctx.enter_context(tc.tile_pool(name="consts", bufs=1))
        kq = ctx.enter_context(tc.tile_pool(name="kq", bufs=5))
        ktp = ctx.enter_context(tc.tile_pool(name="ktp", bufs=3))
        soft = ctx.enter_context(tc.tile_pool(name="soft", bufs=2))
        outp = ctx.enter_context(tc.tile_pool(name="outp", bufs=2))
        small = ctx.enter_context(tc.tile_pool(name="small", bufs=4))
        ps_kt = ctx.enter_context(
            tc.tile_pool(name="ps_kt", bufs=2, space="PSUM"))
        ps_sc = ctx.enter_context(
            tc.tile_pool(name="ps_sc", bufs=2, space="PSUM"))
        ps_pt = ctx.enter_context(
            tc.tile_pool(name="ps_pt", bufs=2, space="PSUM"))
        ps_pv = ctx.enter_context(
            tc.tile_pool(name="ps_pv", bufs=2, space="PSUM"))

        ident = consts.tile([128, 128], F32)
        nc.sync.dma_start(out=ident[:], in_=ident_d.ap())
        es_sb = consts.tile([128, 1], F32)
        nc.sync.dma_start(out=es_sb[:], in_=esink.ap())
        warm = ps_pt.tile([128, 128], F32, tag="ps_pt")
        nc.tensor.transpose(warm[:], ident[:], ident[:])

        def load_token(t):
            idx_sb = kq.tile([128, TOPK // 16], I16, tag="idx")
            nc.sync.dma_start(out=idx_sb[:], in_=idx.ap()[t])
            k_sb = kq.tile([128, NB, D], F32, tag="k")
            nc.gpsimd.dma_gather(
                out_ap=k_sb[:],
                in_ap=kv.ap(),
                idxs_ap=idx_sb[:],
                num_idxs=TOPK,
                num_idxs_reg=TOPK,
                elem_size=D,
            )
            q_sb = kq.tile([128, NCH, H], F32, tag="q")
            nc.sync.dma_start(out=q_sb[:], in_=q_t.ap()[t])
            q_act = kq.tile([128, NCH, H], F32, tag="qa")
            nc.scalar.copy(q_act[:], q_sb[:])
            return k_sb, q_act

        def build_kt(k_sb):
            kt_sb = ktp.tile([128, NCH, TOPK], F32, tag="kt")
            for c in range(NCH):
                pp = 128 if c < 4 else D - 512
                pst = ps_kt.tile([128, TOPK], F32, tag="ps_kt")
                for b in range(NB):
                    nc.tensor.transpose(
                        pst[:pp, b * 128:(b + 1) * 128],
                        k_sb[:, b, c * 128:c * 128 + pp],
                        ident[:],
                    )
                nc.scalar.copy(kt_sb[:pp, c, :], pst[:pp, :])
            return kt_sb

        def pair_body(tA):
            kA, qA = load_token(tA)
            kB, qB = load_token(tA + 1)
            ktA = build_kt(kA)
            ktB = build_kt(kB)

            sc = ps_sc.tile([128, TOPK], F32, tag="sc")
            for c in range(NCH):
                kk = 128 if c < 4 else D - 512
                st, sp = (c == 0), (c == NCH - 1)
                nc.tensor.matmul(
                    sc[0:64, :], lhsT=qA[:kk, c, :], rhs=ktA[:kk, c, :],
                    start=st, stop=sp, tile_position=(0, 0),
                    skip_group_check=True,
                )
                nc.tensor.matmul(
                    sc[64:128, :], lhsT=qB[:kk, c, :], rhs=ktB[:kk, c, :],
                    start=st, stop=sp, tile_position=(0, 64),
                    skip_group_check=True,
                )

            p_sb = soft.tile([128, TOPK], F32, tag="p")
            sum_p = small.tile([128, 1], F32, tag="sum")
            nc.scalar.activation(
                p_sb[:], sc[:], mybir.ActivationFunctionType.Exp,
                accum_out=sum_p[:],
            )
            den = small.tile([128, 1], F32, tag="den")
            nc.vector.tensor_add(den[:], sum_p[:], es_sb[:])
            rec = small.tile([128, 1], F32, tag="rec")
            nc.vector.reciprocal(rec[:], den[:])

            pt_sb = soft.tile([128, NB, 128], F32, tag="pt")
            for b in range(NB):
                pst = ps_pt.tile([128, 128], F32, tag="ps_pt")
                nc.tensor.transpose(
                    pst[:], p_sb[:, b * 128:(b + 1) * 128], ident[:])
                nc.vector.tensor_copy(pt_sb[:, b, :], pst[:])

            pv = ps_pv.tile([128, DV], F32, tag="pv")
            for b in range(NB):
                st, sp = (b == 0), (b == NB - 1)
                nc.tensor.matmul(
                    pv[0:64, :], lhsT=pt_sb[:, b, 0:64], rhs=kA[:, b, 0:DV],
                    start=st, stop=sp, tile_position=(0, 0),
                    skip_group_check=True,
                )
                nc.tensor.matmul(
                    pv[64:128, :], lhsT=pt_sb[:, b, 64:128],
                    rhs=kB[:, b, 0:DV],
                    start=st, stop=sp, tile_position=(0, 64),
                    skip_group_check=True,
                )

            o_sb = outp.tile([128, DV], F32, tag="o")
            nc.vector.tensor_scalar_mul(o_sb[:], pv[:], rec[:])
            nc.sync.dma_start(
                out=out_flat[tA * H:tA * H + 128, :], in_=o_sb[:])

        loop_cm = tc.For_i(0, repeat, 1) if repeat > 1 else nullcontext()
        with loop_cm:
            for i in range(t_loc // 2):
                pair_body(2 * i)

    nc.compile()
    return nc


def build_program(t_loc=T_LOC, repeat=1, mode=MODE):
    if mode == "v2":
        return build_program_v2(t_loc, repeat)
    if mode == "f16":
        return build_program_f16(t_loc, repeat)
    assert mode == "fp32"
    return build_program_fp32(t_loc, repeat)


# ---------------- host-side prep ----------------

def prep_shared(kv_cache, attn_sink, mode=MODE):
    """Per-run, core-independent host prep (replicated to every core)."""
    es = np.exp(np.asarray(attn_sink, np.float64)).astype(np.float32)
    esink = np.ascontiguousarray(np.tile(es, 2)[:, None])
    if mode in ("f16", "v2"):
        kv = np.zeros((NKV, DP), np.float16)
        kv[:, :D] = np.asarray(kv_cache, np.float32).astype(np.float16)
        ident = np.eye(128, dtype=np.float16)
    else:
        kv = np.ascontiguousarray(np.asarray(kv_cache, np.float32))
        ident = np.eye(128, dtype=np.float32)
    return {"kv": kv, "esink": esink, "ident": ident}


def prep_core_inputs(q, shared, topk_indices, core, t_loc=T_LOC, mode=MODE):
    t0 = core * t_loc
    qs = (np.asarray(q[t0:t0 + t_loc]) * SCALE).astype(np.float32)
    qpad = np.zeros((t_loc, H, DP), np.float32)
    qpad[:, :, :D] = qs
    qtr = qpad.reshape(t_loc, H, NCH, 128).transpose(0, 3, 2, 1)
    if mode in ("f16", "v2"):
        q_t = np.ascontiguousarray(qtr.astype(np.float16))
    else:
        q_t = np.ascontiguousarray(qtr)

    tk = np.asarray(topk_indices[t0:t0 + t_loc])
    if SORT_IDX:
        # softmax/PV are order-invariant over the topk axis, so sorting
        # is legal; whether it helps HBM locality is measured, not assumed.
        tk = np.sort(tk, axis=-1)
    tk = tk.astype(np.int16)
    wrap = tk.reshape(t_loc, TOPK // 16, 16).transpose(0, 2, 1)
    idx = np.ascontiguousarray(np.tile(wrap, (1, 8, 1)))

    return {"q_t": q_t, "kv": shared["kv"], "idx": idx,
            "esink": shared["esink"], "ident": shared["ident"]}


_PROGRAM_CACHE = {}


def _get_program(t_loc, mode=MODE):
    key = (t_loc, mode)
    if key not in _PROGRAM_CACHE:
        _PROGRAM_CACHE[key] = build_program(t_loc, mode=mode)
    return _PROGRAM_CACHE[key]


def run(q, kv_cache, topk_indices, attn_sink, trace=False, mode=MODE):
    nc = _get_program(T_LOC, mode)
    shared = prep_shared(kv_cache, attn_sink, mode)
    in_maps = [
        prep_core_inputs(q, shared, topk_indices, c, mode=mode)
        for c in range(N_CORES)
    ]
    res = run_bass_kernel_spmd(nc, in_maps, list(range(N_CORES)),
                               trace=trace)
    out = np.concatenate([res.results[c]["out"] for c in range(N_CORES)],
                         axis=0)
    return out, res


def kernel(q, kv_cache, topk_indices, attn_sink):
    out, _ = run(q, kv_cache, topk_indices, attn_sink, trace=False)
    return out.astype(np.float32)

